# revision 41
# baseline (speedup 1.0000x reference)
"""BitSelfAttention on 8 TRN2 NeuronCores, fp8-DoubleRow edition.

Sharding: core c handles batch b = c//2 and head-group hg = c%2 (8 of 16 heads).
Each core computes its 8 heads' QKV projections + causal attention + its slice
of the o_proj GEMM, producing a partial output (transposed, [D, T], fp32).
Host: splits BitLinear weights into exact ternary signs (fp8) and gamma scales,
pre-transposes operands into matmul-friendly layouts, and sums the two
head-group partials per batch at the end.

Precision plan (validated in numpy sim to rel ~4e-3 vs fp32 reference):
  - QKV projections run as fp8 DoubleRow matmuls (2 fp8 MACs/cell/cycle):
    x in fp8 e4m3, weights are the exact ternary signs in fp8; gamma scales
    are folded out (gq*gk into the softmax exp scale, gv*go into wo).
  - PV attention matmuls run fp8: V^T tiles and P (exp output) in fp8,
    paired key tiles via DoubleRow for the non-diagonal blocks.
  - Early tokens are precision-patched (fp8 noise does not average out over
    few keys): q/k/v for tokens 0..127 are recomputed from bf16 x via
    mixed-dtype matmuls (fp8 weights x bf16 rhs) overwriting the fp8-derived
    values, and the qb0/kt0 attention tile (queries 0..511 x keys 0..127)
    keeps P and V^T in bf16.
  - S = K^T.T @ Q stays bf16 (128-deep contraction gains nothing from
    DoubleRow). o_proj runs local heads 0,1 in fp8 (one DoubleRow matmul
    replaces two bf16; weights are sign*2^-6 exactly, with the go*gv*64
    counter-scale folded into those heads' row-sum reduction) and heads
    2..7 in bf16: full-fp8 o_proj input quantization would cost ~2.7e-2
    rel error, the quarter-fp8 split lands at 1.4e-2 (gate is 2e-2).

Device layouts (per core):
  xT8  [D, T]   fp8  : x[b].T                  (rhs for Q/K/V^T projections)
  xTb  [D, 128] bf16 : x[b].T, first 128 tokens (patch rhs)
  wqT  [H, 128, KD*128] fp8 : ternary signs of w_q[hg-rows], pre-tiled
  wkT/wvT likewise; woT [MT, 128, H*128] bf16 : (go*gv)*sign(w_o)[:, hg-cols]
  cmask[4, 128, 512] fp8, cmb [128, 128] bf16 : causal masks
  qks  [128, 1] f32  : gq*gk/sqrt(dh), the exp scale (runtime data, not baked)
  outT [D, T]  fp32  : partial output, transposed

Attention per head: S^T = K^T_tile.T @ Q^T_block so softmax rows land on the
free axis; P^T = exp(S^T*qks) (ACT, PSUM->fp8); key-tile partial row-sums
accumulate in fp16 on the vector engine (16-bit DVE ops run 2x, and fp16 is a
valid matmul dtype so the all-ones partition-reduction consumes it directly);
O^T accumulates V_tile.T @ P^T over key tiles (DoubleRow pairs off-diagonal,
narrowed fp8 singles on the diagonal); normalize with
fast-reciprocal+multiply during PSUM eviction.
o_proj consumes O^T tiles as stationary operands; its per-token-block chains
double as PE fill work zipped into the last head's attention, just as each
head's projection chains are zipped into the previous head's attention (the
attention inner loop is otherwise exp-latency-gated on the in-order PE).
"""

import math

import ml_dtypes
import numpy as np

import concourse.mybir as mybir
import concourse.tile as tile
from concourse import bacc
from concourse import bass_utils
from concourse.masks import make_identity

BF16 = mybir.dt.bfloat16
FP16 = mybir.dt.float16
F8 = mybir.dt.float8e4
F32 = mybir.dt.float32
DR = mybir.MatmulPerfMode.DoubleRow

D_MODEL = 2048
N_HEAD = 16
D_HEAD = 128
B = 4
T_FULL = 2048
N_CORES = 8
F_LOC = D_MODEL // 2  # features per core (8 heads)


def build_bass(T=T_FULL, D=D_MODEL, F=F_LOC, debug=False):
    """Build the single-core program (SPMD across 8 cores via input data)."""
    P = 128
    KD = D // P      # contraction 128-tiles
    TT = T // P      # token 128-tiles
    TB = T // 512    # token 512-blocks
    H = F // P       # local heads
    MT = D // P      # output-dmodel 128-tiles
    KT_PER_B = 512 // P

    nc = bacc.Bacc("TRN2", target_bir_lowering=False, debug=debug,
                   num_devices=N_CORES)
    xT_d = nc.dram_tensor("xT8", [D, T], F8, kind="ExternalInput").ap()
    xTb_d = nc.dram_tensor("xTb", [D, P], BF16, kind="ExternalInput").ap()
    wqT_d = nc.dram_tensor("wqT", [H, P, KD * P], F8,
                           kind="ExternalInput").ap()
    wkT_d = nc.dram_tensor("wkT", [H, P, KD * P], F8,
                           kind="ExternalInput").ap()
    wvT_d = nc.dram_tensor("wvT", [H, P, KD * P], F8,
                           kind="ExternalInput").ap()
    woT_d = nc.dram_tensor("woT", [MT, P, H * P], BF16,
                           kind="ExternalInput").ap()
    # fp8 o_proj weights for local heads 0,1: sign * 2^-6 (the e4m3 min
    # normal — exact); the go*gv*64 counter-scale is folded into those
    # heads' row-sum reduction so their fp8 ot values stay in normal range
    # and the PSUM contributions match the bf16 chunks' scale.
    woT8_d = nc.dram_tensor("woT8", [MT, P, 2 * P], F8,
                            kind="ExternalInput").ap()
    onesgv_d = nc.dram_tensor("onesgv", [P, P], BF16,
                              kind="ExternalInput").ap()
    cm_d = nc.dram_tensor("cmask", [4, P, 512], F8, kind="ExternalInput").ap()
    cmb_d = nc.dram_tensor("cmb", [P, P], BF16, kind="ExternalInput").ap()
    qks_d = nc.dram_tensor("qks", [P, 1], F32, kind="ExternalInput").ap()
    out_d = nc.dram_tensor("outT", [D, T], F32, kind="ExternalOutput").ap()

    with tile.TileContext(nc) as tc:
        with (
            tc.tile_pool(name="big", bufs=1) as big,
            tc.tile_pool(name="work", bufs=2) as work,
            tc.tile_pool(name="psS", bufs=3, space="PSUM") as psS,
            tc.tile_pool(name="psO", bufs=2, space="PSUM") as psO,
            tc.tile_pool(name="psR", bufs=1, space="PSUM") as psR,
            tc.tile_pool(name="psP", bufs=2, space="PSUM") as psP,
        ):
            # ---- persistent inputs (head-0 weights first: first MMs need them)
            wvh0 = work.tile([P, KD, P], F8, name="wvh0", tag="wvh")
            nc.sync.dma_start(out=wvh0.rearrange("p kd f -> p (kd f)"),
                              in_=wvT_d[0])
            ones = big.tile([P, P], BF16, name="ones_sb", tag="ones", bufs=1)
            nc.vector.memset(ones, 1.0)
            onesgv = big.tile([P, P], BF16, name="onesgv_sb", tag="onesgv",
                              bufs=1)
            nc.sync.dma_start(out=onesgv, in_=onesgv_d)
            ident = big.tile([P, P], BF16, name="ident_sb", tag="ident", bufs=1)
            make_identity(nc, ident)
            qks = big.tile([P, 1], F32, name="qks_sb", tag="qks", bufs=1)
            nc.sync.dma_start(out=qks, in_=qks_d)
            # x lives as KD/2 independent kd-PAIR tiles: tile readiness is
            # per-tile, so the first DoubleRow matmul waits for 2 DMAs, not
            # all 16 (one monolithic [P, KD, T] tile stalled the PE ~10us at
            # startup). Finer per-DMA chunking measures WORSE (~600ns of
            # queue time per descriptor), and routing half onto the Scalar
            # hwdge queue produced NaNs.
            xt8 = [big.tile([P, 2, T], F8, name=f"xt8_{k2}", tag="xt8",
                            bufs=KD // 2) for k2 in range(KD // 2)]
            for k2 in range(KD // 2):
                for j in (0, 1):
                    kd = 2 * k2 + j
                    nc.sync.dma_start(out=xt8[k2][:, j, :],
                                      in_=xT_d[kd * P:(kd + 1) * P, :])
            xtb = big.tile([P, KD, P], BF16, name="xtb", tag="xtb", bufs=1)
            for kd in range(KD):
                nc.sync.dma_start(out=xtb[:, kd, :],
                                  in_=xTb_d[kd * P:(kd + 1) * P, :])
            cmask = big.tile([P, 4, 512], F8, name="cmask_sb", tag="cmask",
                             bufs=1)
            for i in range(4):
                nc.sync.dma_start(out=cmask[:, i, :], in_=cm_d[i])
            cmb = big.tile([P, P], BF16, name="cmb_sb", tag="cmb", bufs=1)
            nc.sync.dma_start(out=cmb, in_=cmb_d)
            # heads 0,1 keep their normalized attention in ONE fp8 pair tile
            # (adjacent for the o_proj DoubleRow rhs); heads 2+ stay bf16
            ot8 = big.tile([P, 2, T], F8, name="ot8", tag="ot8", bufs=1)
            ot = [big.tile([P, T], BF16, name=f"ot{h}", tag="ot", bufs=H - 2)
                  for h in range(2, H)]
            ot = [None, None] + ot

            # ---- per-head pipeline with cross-head fill interleaving.
            # The attention inner loop is ACT(exp)-gated; we pump projection
            # matmuls of the NEXT head between attention iterations so the
            # (in-order) PE always has fill work.
            def load_head_weights(h, wvh=None):
                if wvh is None:
                    wvh = work.tile([P, KD, P], F8, name=f"wvh{h}",
                                    tag="wvh")
                    nc.sync.dma_start(out=wvh.rearrange("p kd f -> p (kd f)"),
                                      in_=wvT_d[h])
                wqh = work.tile([P, KD, P], F8, name=f"wqh{h}", tag="wqh")
                nc.sync.dma_start(out=wqh.rearrange("p kd f -> p (kd f)"),
                                  in_=wqT_d[h])
                wkh = work.tile([P, KD, P], F8, name=f"wkh{h}", tag="wkh")
                nc.sync.dma_start(out=wkh.rearrange("p kd f -> p (kd f)"),
                                  in_=wkT_d[h])
                return wqh, wkh, wvh

            def load_qk_weights(h):
                wqh = work.tile([P, KD, P], F8, name=f"wqh{h}", tag="wqh")
                nc.sync.dma_start(out=wqh.rearrange("p kd f -> p (kd f)"),
                                  in_=wqT_d[h])
                wkh = work.tile([P, KD, P], F8, name=f"wkh{h}", tag="wkh")
                nc.sync.dma_start(out=wkh.rearrange("p kd f -> p (kd f)"),
                                  in_=wkT_d[h])
                return wqh, wkh

            def alloc_head_tiles(h):
                vT = work.tile([P, T], BF16, name=f"vT{h}", tag="vT")
                vh = work.tile([P, TT, P], F8, name=f"vh{h}", tag="vh")
                vb0 = work.tile([P, P], BF16, name=f"vb0_{h}", tag="vb0")
                qt_ = work.tile([P, T], BF16, name=f"qt{h}", tag="qt")
                kt_ = work.tile([P, T], BF16, name=f"kt{h}", tag="kt")
                return vT, vh, vb0, qt_, kt_

            def proj_fill_gen(ws, tiles):
                """V^T then Q^T then K^T projection chains (fp8 DoubleRow),
                then the bf16 early-token patch chains, yielding after every
                matmul so the caller can interleave them."""
                wqh, wkh, wvh = ws
                vT, vh, vb0, qt_, kt_ = tiles
                for wh, dst in ((wvh, vT), (wqh, qt_), (wkh, kt_)):
                    for tb in range(TB):
                        ts_ = slice(tb * 512, (tb + 1) * 512)
                        ps = psP.tile([P, 512], F32, name="psfill", tag="psp")
                        for k2 in range(KD // 2):
                            nc.tensor.matmul(ps,
                                             lhsT=wh[:, 2 * k2:2 * k2 + 2, :],
                                             rhs=xt8[k2][:, :, ts_],
                                             start=(k2 == 0),
                                             stop=(k2 == KD // 2 - 1),
                                             perf_mode=DR)
                            yield
                        nc.vector.tensor_copy(out=dst[:, ts_], in_=ps)
                        if tb == 0:
                            # early-token patch: recompute tokens 0..127 from
                            # bf16 x (mixed-dtype: fp8 sign weights x bf16
                            # rhs), overwriting the fp8-derived values right
                            # after the tb0 eviction so the patch lands well
                            # before the head barrier (its eviction otherwise
                            # stalls the first S matmul of the next head).
                            ps2 = psP.tile([P, 512], F32, name="psfill",
                                           tag="psp")
                            for kd in range(KD):
                                nc.tensor.matmul(ps2[:, 0:P],
                                                 lhsT=wh[:, kd, :],
                                                 rhs=xtb[:, kd, :],
                                                 start=(kd == 0),
                                                 stop=(kd == KD - 1))
                                yield
                            nc.vector.tensor_copy(out=dst[:, 0:P],
                                                  in_=ps2[:, 0:P])

            def pump(gen, n):
                for _ in range(n):
                    try:
                        next(gen)
                    except StopIteration:
                        return False
                return True

            def pump_n(gen, n):
                c = 0
                for _ in range(n):
                    try:
                        next(gen)
                        c += 1
                    except StopIteration:
                        break
                return c

            def oproj_nb_gen(nb):
                """o_proj chains for one token block (needs all heads' ot
                columns of that block only), yielding per matmul."""
                ns = slice(nb * 512, (nb + 1) * 512)
                for m in range(MT):
                    ps = psP.tile([P, 512], F32, name="psout", tag="psp")
                    # heads 0,1: one fp8 DoubleRow matmul replaces two bf16
                    nc.tensor.matmul(ps, lhsT=wo8all[:, m, :, :],
                                     rhs=ot8[:, :, ns],
                                     start=True, stop=False, perf_mode=DR,
                                     skip_group_check=True)
                    yield
                    for hh in range(2, H):
                        nc.tensor.matmul(ps, lhsT=woall[:, m, hh, :],
                                         rhs=ot[hh][:, ns],
                                         start=False, stop=(hh == H - 1),
                                         skip_group_check=True)
                        yield
                    stg = work.tile([P, 512], F32, name="ostage", tag="ostage",
                                    bufs=6)
                    nc.vector.tensor_copy(out=stg, in_=ps)
                    nc.sync.dma_start(out=out_d[m * P:(m + 1) * P, ns],
                                      in_=stg)

            # head-0 Q/K weights and head-1 weights load after xt (the V^T
            # chains consume xt first; the Q chains run ~4 chain-times later)
            ws_list = [None] * (H + 2)
            wqh0, wkh0 = load_qk_weights(0)
            ws_list[0] = (wqh0, wkh0, wvh0)
            if H > 1:
                ws_list[1] = load_head_weights(1)
            cur_tiles = alloc_head_tiles(0)
            g0 = proj_fill_gen(ws_list[0], cur_tiles)
            while pump(g0, 1):
                pass

            fills = []

            def pump_fills(n):
                while n > 0 and fills:
                    n -= pump_n(fills[0], n)
                    if n > 0:
                        fills.pop(0)

            # o_proj weights are SBUF-resident: 64 per-(block,m-tile) weight
            # DMAs during the last head's attention saturate the Sync queue
            # (which also carries the output writes). Loaded around head 2,
            # when the DMA queue is otherwise quiet.
            woall = big.tile([P, MT, H, P], BF16, name="woall", tag="woall",
                             bufs=1)
            wo8all = big.tile([P, MT, 2, P], F8, name="wo8all", tag="wo8all",
                              bufs=1)

            for h in range(H):
                if h == 2:
                    for m in range(MT):
                        nc.sync.dma_start(
                            out=woall[:, m, :, :].rearrange("p h f -> p (h f)"),
                            in_=woT_d[m])
                        nc.sync.dma_start(
                            out=wo8all[:, m, :, :].rearrange("p h f -> p (h f)"),
                            in_=woT8_d[m])
                vT, vh, vb0, qt_, kt_ = cur_tiles
                # prefetch weights two heads ahead so fill matmuls never
                # wait on their DMA (a blocked fill stalls the in-order PE)
                if h + 2 < H:
                    ws_list[h + 2] = load_head_weights(h + 2)
                if h + 1 < H:
                    next_tiles = alloc_head_tiles(h + 1)
                    fills.append(proj_fill_gen(ws_list[h + 1], next_tiles))
                else:
                    next_tiles = None

                def emit_transpose(kt):
                    # lives in the psS pool: psP slots are held long by
                    # in-flight interleaved fill chains
                    pst = psS.tile([P, 512], BF16, name="pst", tag="pss")
                    nc.tensor.transpose(pst[:, 0:P],
                                        vT[:, kt * P:(kt + 1) * P], ident)
                    nc.vector.tensor_copy(out=vh[:, kt, :], in_=pst[:, 0:P])
                    if kt == 0:
                        # bf16 copy of key-tile 0 for the qb0 precision patch
                        nc.vector.tensor_copy(out=vb0, in_=pst[:, 0:P])

                # causal attention, S^T layout (keys on partitions).
                # Off-diagonal key tiles are paired into fp8 DoubleRow PV
                # matmuls; diagonal tiles (kt = 4*qb+di) only contribute to
                # query columns >= 128*di of the block and stay narrowed fp8
                # singles. The qb0/kt0 tile runs fully in bf16 (early-token
                # precision). Only the first 128 columns of a (narrowed)
                # diagonal tile are triangular; the rest are fully allowed.
                for qb in range(TB):
                    nkt = KT_PER_B * (qb + 1)
                    for kt in range(KT_PER_B * qb, nkt):
                        emit_transpose(kt)
                    psO_t = psO.tile([P, 512], F32, name="psodt", tag="pso")
                    # fp16 row-sum accumulator: 16-bit DVE ops run 2x, and
                    # fp16 is a valid matmul dtype so the ones-reduction can
                    # consume it directly (no bf16 staging copy). ~0.05% per
                    # add, ~0.2% on the normalizer over 16 adds: negligible.
                    racc = work.tile([P, 512], FP16, name="racc", tag="racc")
                    # off-diagonal pairs (kt in [0, 4qb), even count)
                    for pr in range(KT_PER_B * qb // 2):
                        ptp = work.tile([P, 2, 512], F8, name="ptp",
                                        tag="ptp", bufs=8)
                        for i in (0, 1):
                            kt = 2 * pr + i
                            psS_t = psS.tile([P, 512], F32, name="pssc",
                                             tag="pss")
                            nc.tensor.matmul(psS_t,
                                             lhsT=kt_[:, kt * P:(kt + 1) * P],
                                             rhs=qt_[:, qb * 512:(qb + 1) * 512],
                                             start=True, stop=True)
                            nc.scalar.activation(
                                out=ptp[:, i, :], in_=psS_t,
                                func=mybir.ActivationFunctionType.Exp,
                                scale=qks)
                            if kt == 0:
                                nc.vector.tensor_copy(out=racc,
                                                      in_=ptp[:, 0, :])
                            else:
                                nc.vector.tensor_add(racc, racc,
                                                     ptp[:, i, :])
                            pump_fills(2)
                        nc.tensor.matmul(psO_t, lhsT=vh[:, 2 * pr:2 * pr + 2, :],
                                         rhs=ptp,
                                         start=(pr == 0), stop=False,
                                         perf_mode=DR, skip_group_check=True)
                        pump_fills(1)
                    # diagonal singles
                    for di in range(KT_PER_B):
                        kt = KT_PER_B * qb + di
                        c0 = di * P  # first live query column
                        w = 512 - c0
                        qs = slice(qb * 512 + c0, (qb + 1) * 512)
                        psS_t = psS.tile([P, 512], F32, name="pssc", tag="pss")
                        nc.tensor.matmul(psS_t[:, :w],
                                         lhsT=kt_[:, kt * P:(kt + 1) * P],
                                         rhs=qt_[:, qs],
                                         start=True, stop=True)
                        if qb == 0 and kt == 0:
                            pt0 = work.tile([P, 512], BF16, name="pt0",
                                            tag="pt0", bufs=1)
                            nc.scalar.activation(
                                out=pt0, in_=psS_t,
                                func=mybir.ActivationFunctionType.Exp,
                                scale=qks)
                            nc.vector.tensor_mul(pt0[:, :P], pt0[:, :P], cmb)
                            pump_fills(2)  # PE fill work while exp runs
                            nc.tensor.matmul(psO_t, lhsT=vb0, rhs=pt0,
                                             start=True, stop=(di == KT_PER_B - 1),
                                             skip_group_check=True)
                            nc.vector.tensor_copy(out=racc, in_=pt0)
                        else:
                            pt = work.tile([P, 512], F8, name="pexp", tag="pt",
                                           bufs=10)
                            nc.scalar.activation(
                                out=pt[:, :w], in_=psS_t[:, :w],
                                func=mybir.ActivationFunctionType.Exp,
                                scale=qks)
                            # first 128 cols of every narrowed diagonal
                            # window are triangular
                            nc.vector.tensor_mul(pt[:, :P], pt[:, :P],
                                                 cmask[:, 0, :P])
                            pump_fills(2)  # PE fill work while exp runs
                            nc.tensor.matmul(psO_t[:, c0:], lhsT=vh[:, kt, :],
                                             rhs=pt[:, :w],
                                             start=(qb == 0 and di == 0),
                                             stop=(di == KT_PER_B - 1),
                                             skip_group_check=True)
                            if kt == 0:
                                nc.vector.tensor_copy(out=racc, in_=pt)
                            else:
                                nc.vector.tensor_add(racc[:, c0:], racc[:, c0:],
                                                     pt[:, :w])
                        pump_fills(di & 1)
                    psR_t = psR.tile([P, 512], F32, name="psrow", tag="psr")
                    nc.tensor.matmul(psR_t, lhsT=(onesgv if h < 2 else ones),
                                     rhs=racc, start=True, stop=True)
                    rec = work.tile([P, 512], F32, name="rec", tag="rec")
                    nc.vector.reciprocal_approx_fast(out=rec, in_=psR_t)
                    if h < 2:
                        nc.vector.tensor_mul(
                            ot8[:, h, qb * 512:(qb + 1) * 512], psO_t, rec)
                    else:
                        nc.vector.tensor_mul(
                            ot[h][:, qb * 512:(qb + 1) * 512], psO_t, rec)
                    if h == H - 1:
                        # this token block's ot columns are now complete for
                        # every head: its o_proj chains become fill work
                        fills.append(oproj_nb_gen(qb))
                    pump_fills(4)
                if h < H - 1:
                    # finish next head's projections before its attention
                    while fills:
                        pump_fills(64)
                cur_tiles = next_tiles
            # drain remaining o_proj work
            while fills:
                pump_fills(64)

    nc.compile()
    return nc


def _bitlinear_sign_gamma(w):
    """BitLinear split: exact ternary sign pattern and the gamma scale."""
    w = np.asarray(w, dtype=np.float32)
    gamma = max(np.float32(np.abs(w).mean(dtype=np.float32)),
                np.float32(1e-5))
    sign = np.clip(np.round(w / gamma), -1.0, 1.0).astype(np.float32)
    return sign, gamma


def _causal_masks():
    k = np.arange(128)[:, None]
    q = np.arange(512)[None, :]
    m = np.stack([(k <= q - 128 * i) for i in range(4)]).astype(np.float32)
    return m


def _tile_qkv(w_shard):
    """[F, D] -> [H, 128, KD*128]: [h, p, kd*128+f] = w_shard[h*128+f, kd*128+p]."""
    Fs, Ds = w_shard.shape
    a = w_shard.reshape(Fs // 128, 128, Ds // 128, 128)  # [h, f, kd, p]
    a = a.transpose(0, 3, 2, 1).reshape(Fs // 128, 128, Ds)
    return np.ascontiguousarray(a)


def _tile_wo(wo_shard):
    """[D, F] -> [MT, 128, H*128]: [m, p, h*128+j] = wo_shard[m*128+j, h*128+p]."""
    Ds, Fs = wo_shard.shape
    a = wo_shard.reshape(Ds // 128, 128, Fs // 128, 128)  # [m, j, h, p]
    a = a.transpose(0, 3, 2, 1).reshape(Ds // 128, 128, Fs)
    return np.ascontiguousarray(a)


def _prep_inputs(x, wq, wk, wv, wo):
    bf = ml_dtypes.bfloat16
    f8 = ml_dtypes.float8_e4m3fn
    x = np.asarray(x, dtype=np.float32)
    sq, gq = _bitlinear_sign_gamma(wq)
    sk, gk = _bitlinear_sign_gamma(wk)
    sv, gv = _bitlinear_sign_gamma(wv)
    so, go = _bitlinear_sign_gamma(wo)
    cm = _causal_masks()
    cmask = cm.astype(f8)
    cmb = np.ascontiguousarray(cm[0][:, :128]).astype(bf)
    qks = np.full((128, 1), gq * gk / math.sqrt(D_HEAD), dtype=np.float32)
    onesgv = np.full((128, 128), 1.0 / (go * gv * 64.0),
                     dtype=np.float32).astype(bf)
    so_8 = so * np.float32(2.0 ** -6)
    xT8s = [np.ascontiguousarray(x[b].T).astype(f8) for b in range(B)]
    xTbs = [np.ascontiguousarray(x[b].T[:, :128]).astype(bf) for b in range(B)]
    wo_scaled = so * (go * gv)
    shards = []
    for hg in range(2):
        rows = slice(hg * F_LOC, (hg + 1) * F_LOC)
        shards.append({
            "wqT": _tile_qkv(sq[rows, :]).astype(f8),
            "wkT": _tile_qkv(sk[rows, :]).astype(f8),
            "wvT": _tile_qkv(sv[rows, :]).astype(f8),
            "woT": _tile_wo(wo_scaled[:, rows]).astype(bf),
            "woT8": _tile_wo(
                so_8[:, hg * F_LOC:hg * F_LOC + 256]).astype(f8),
        })
    in_maps = []
    for c in range(N_CORES):
        b, hg = c // 2, c % 2
        m = {"xT8": xT8s[b], "xTb": xTbs[b], "cmask": cmask, "cmb": cmb,
             "qks": qks, "onesgv": onesgv}
        m.update(shards[hg])
        in_maps.append(m)
    return in_maps


_NC_CACHE = {}


def _get_nc():
    if "nc" not in _NC_CACHE:
        _NC_CACHE["nc"] = build_bass()
    return _NC_CACHE["nc"]


def run(x, wq, wk, wv, wo, trace=False):
    nc = _get_nc()
    in_maps = _prep_inputs(x, wq, wk, wv, wo)
    res = bass_utils.run_bass_kernel_spmd(
        nc, in_maps, core_ids=list(range(N_CORES)), trace=trace)
    out = np.empty((B, T_FULL, D_MODEL), dtype=np.float32)
    for b in range(B):
        out[b] = (res.results[2 * b]["outT"]
                  + res.results[2 * b + 1]["outT"]).T
    return out, res


def kernel(x, wq, wk, wv, wo):
    out, _ = run(x, wq, wk, wv, wo)
    return out


# revision 42
# speedup vs baseline: 1.0098x; 1.0098x over previous
"""BitSelfAttention on 8 TRN2 NeuronCores, fp8-DoubleRow edition.

Sharding: core c handles batch b = c//2 and head-group hg = c%2 (8 of 16 heads).
Each core computes its 8 heads' QKV projections + causal attention + its slice
of the o_proj GEMM, producing a partial output (transposed, [D, T], fp32).
Host: splits BitLinear weights into exact ternary signs (fp8) and gamma scales,
pre-transposes operands into matmul-friendly layouts, and sums the two
head-group partials per batch at the end.

Precision plan (validated in numpy sim to rel ~4e-3 vs fp32 reference):
  - QKV projections run as fp8 DoubleRow matmuls (2 fp8 MACs/cell/cycle):
    x in fp8 e4m3, weights are the exact ternary signs in fp8; gamma scales
    are folded out (gq*gk into the softmax exp scale, gv*go into wo).
  - PV attention matmuls run fp8: V^T tiles and P (exp output) in fp8,
    paired key tiles via DoubleRow for the non-diagonal blocks.
  - Early tokens are precision-patched (fp8 noise does not average out over
    few keys): q/k/v for tokens 0..127 are recomputed from bf16 x via
    mixed-dtype matmuls (fp8 weights x bf16 rhs) overwriting the fp8-derived
    values, and the qb0/kt0 attention tile (queries 0..511 x keys 0..127)
    keeps P and V^T in bf16.
  - S = K^T.T @ Q stays bf16 (128-deep contraction gains nothing from
    DoubleRow). o_proj runs local heads 0,1 in fp8 (one DoubleRow matmul
    replaces two bf16; weights are sign*2^-6 exactly, with the go*gv*64
    counter-scale folded into those heads' row-sum reduction) and heads
    2..7 in bf16: full-fp8 o_proj input quantization would cost ~2.7e-2
    rel error, the quarter-fp8 split lands at 1.4e-2 (gate is 2e-2).

Device layouts (per core):
  xT8  [D, T]   fp8  : x[b].T                  (rhs for Q/K/V^T projections)
  xTb  [D, 128] bf16 : x[b].T, first 128 tokens (patch rhs)
  wqT  [H, 128, KD*128] fp8 : ternary signs of w_q[hg-rows], pre-tiled
  wkT/wvT likewise; woT [MT, 128, H*128] bf16 : (go*gv)*sign(w_o)[:, hg-cols]
  cmask[4, 128, 512] fp8, cmb [128, 128] bf16 : causal masks
  qks  [128, 1] f32  : gq*gk/sqrt(dh), the exp scale (runtime data, not baked)
  outT [D, T]  fp32  : partial output, transposed

Attention per head: S^T = K^T_tile.T @ Q^T_block so softmax rows land on the
free axis; P^T = exp(S^T*qks) (ACT, PSUM->fp8); key-tile partial row-sums
accumulate in fp16 on the vector engine (16-bit DVE ops run 2x, and fp16 is a
valid matmul dtype so the all-ones partition-reduction consumes it directly);
O^T accumulates V_tile.T @ P^T over key tiles (DoubleRow pairs off-diagonal,
narrowed fp8 singles on the diagonal); normalize with
fast-reciprocal+multiply during PSUM eviction.
o_proj consumes O^T tiles as stationary operands; its per-token-block chains
double as PE fill work zipped into the last head's attention, just as each
head's projection chains are zipped into the previous head's attention (the
attention inner loop is otherwise exp-latency-gated on the in-order PE).
"""

import math

import ml_dtypes
import numpy as np

import concourse.mybir as mybir
import concourse.tile as tile
from concourse import bacc
from concourse import bass_utils
from concourse.masks import make_identity

BF16 = mybir.dt.bfloat16
FP16 = mybir.dt.float16
F8 = mybir.dt.float8e4
F32 = mybir.dt.float32
DR = mybir.MatmulPerfMode.DoubleRow

D_MODEL = 2048
N_HEAD = 16
D_HEAD = 128
B = 4
T_FULL = 2048
N_CORES = 8
F_LOC = D_MODEL // 2  # features per core (8 heads)


def build_bass(T=T_FULL, D=D_MODEL, F=F_LOC, debug=False):
    """Build the single-core program (SPMD across 8 cores via input data)."""
    P = 128
    KD = D // P      # contraction 128-tiles
    TT = T // P      # token 128-tiles
    TB = T // 512    # token 512-blocks
    H = F // P       # local heads
    MT = D // P      # output-dmodel 128-tiles
    KT_PER_B = 512 // P

    nc = bacc.Bacc("TRN2", target_bir_lowering=False, debug=debug,
                   num_devices=N_CORES)
    xT_d = nc.dram_tensor("xT8", [D, T], F8, kind="ExternalInput").ap()
    xTb_d = nc.dram_tensor("xTb", [D, P], BF16, kind="ExternalInput").ap()
    wqT_d = nc.dram_tensor("wqT", [H, P, KD * P], F8,
                           kind="ExternalInput").ap()
    wkT_d = nc.dram_tensor("wkT", [H, P, KD * P], F8,
                           kind="ExternalInput").ap()
    wvT_d = nc.dram_tensor("wvT", [H, P, KD * P], F8,
                           kind="ExternalInput").ap()
    woT_d = nc.dram_tensor("woT", [MT, P, H * P], BF16,
                           kind="ExternalInput").ap()
    # fp8 o_proj weights for local heads 0,1: sign * 2^-6 (the e4m3 min
    # normal — exact); the go*gv*64 counter-scale is folded into those
    # heads' row-sum reduction so their fp8 ot values stay in normal range
    # and the PSUM contributions match the bf16 chunks' scale.
    woT8_d = nc.dram_tensor("woT8", [MT, P, 2 * P], F8,
                            kind="ExternalInput").ap()
    onesgv_d = nc.dram_tensor("onesgv", [P, P], BF16,
                              kind="ExternalInput").ap()
    cm_d = nc.dram_tensor("cmask", [4, P, 512], F8, kind="ExternalInput").ap()
    cmb_d = nc.dram_tensor("cmb", [P, P], BF16, kind="ExternalInput").ap()
    qks_d = nc.dram_tensor("qks", [P, 1], F32, kind="ExternalInput").ap()
    out_d = nc.dram_tensor("outT", [D, T], F32, kind="ExternalOutput").ap()

    with tile.TileContext(nc) as tc:
        with (
            tc.tile_pool(name="big", bufs=1) as big,
            tc.tile_pool(name="work", bufs=2) as work,
            tc.tile_pool(name="psS", bufs=3, space="PSUM") as psS,
            tc.tile_pool(name="psO", bufs=2, space="PSUM") as psO,
            tc.tile_pool(name="psR", bufs=1, space="PSUM") as psR,
            tc.tile_pool(name="psP", bufs=2, space="PSUM") as psP,
        ):
            # ---- persistent inputs (head-0 weights first: first MMs need them)
            wvh0 = work.tile([P, KD, P], F8, name="wvh0", tag="wvh")
            nc.sync.dma_start(out=wvh0.rearrange("p kd f -> p (kd f)"),
                              in_=wvT_d[0])
            ones = big.tile([P, P], BF16, name="ones_sb", tag="ones", bufs=1)
            nc.vector.memset(ones, 1.0)
            onesgv = big.tile([P, P], BF16, name="onesgv_sb", tag="onesgv",
                              bufs=1)
            nc.sync.dma_start(out=onesgv, in_=onesgv_d)
            ident = big.tile([P, P], BF16, name="ident_sb", tag="ident", bufs=1)
            make_identity(nc, ident)
            qks = big.tile([P, 1], F32, name="qks_sb", tag="qks", bufs=1)
            nc.sync.dma_start(out=qks, in_=qks_d)
            # x lives as KD/2 independent kd-PAIR tiles: tile readiness is
            # per-tile, so the first DoubleRow matmul waits for 2 DMAs, not
            # all 16 (one monolithic [P, KD, T] tile stalled the PE ~10us at
            # startup). Finer per-DMA chunking measures WORSE (~600ns of
            # queue time per descriptor), and routing half onto the Scalar
            # hwdge queue produced NaNs.
            xt8 = [big.tile([P, 2, T], F8, name=f"xt8_{k2}", tag="xt8",
                            bufs=KD // 2) for k2 in range(KD // 2)]
            for k2 in range(KD // 2):
                for j in (0, 1):
                    kd = 2 * k2 + j
                    nc.sync.dma_start(out=xt8[k2][:, j, :],
                                      in_=xT_d[kd * P:(kd + 1) * P, :])
            xtb = big.tile([P, KD, P], BF16, name="xtb", tag="xtb", bufs=1)
            for kd in range(KD):
                nc.sync.dma_start(out=xtb[:, kd, :],
                                  in_=xTb_d[kd * P:(kd + 1) * P, :])
            cmask = big.tile([P, 4, 512], F8, name="cmask_sb", tag="cmask",
                             bufs=1)
            for i in range(4):
                nc.sync.dma_start(out=cmask[:, i, :], in_=cm_d[i])
            cmb = big.tile([P, P], BF16, name="cmb_sb", tag="cmb", bufs=1)
            nc.sync.dma_start(out=cmb, in_=cmb_d)
            # heads 0,1 keep their normalized attention in ONE fp8 pair tile
            # (adjacent for the o_proj DoubleRow rhs); heads 2+ stay bf16
            ot8 = big.tile([P, 2, T], F8, name="ot8", tag="ot8", bufs=1)
            ot = [big.tile([P, T], BF16, name=f"ot{h}", tag="ot", bufs=H - 2)
                  for h in range(2, H)]
            ot = [None, None] + ot

            # ---- per-head pipeline with cross-head fill interleaving.
            # The attention inner loop is ACT(exp)-gated; we pump projection
            # matmuls of the NEXT head between attention iterations so the
            # (in-order) PE always has fill work.
            def load_head_weights(h, wvh=None):
                if wvh is None:
                    wvh = work.tile([P, KD, P], F8, name=f"wvh{h}",
                                    tag="wvh")
                    nc.sync.dma_start(out=wvh.rearrange("p kd f -> p (kd f)"),
                                      in_=wvT_d[h])
                wqh = work.tile([P, KD, P], F8, name=f"wqh{h}", tag="wqh")
                nc.sync.dma_start(out=wqh.rearrange("p kd f -> p (kd f)"),
                                  in_=wqT_d[h])
                wkh = work.tile([P, KD, P], F8, name=f"wkh{h}", tag="wkh")
                nc.sync.dma_start(out=wkh.rearrange("p kd f -> p (kd f)"),
                                  in_=wkT_d[h])
                return wqh, wkh, wvh

            def load_qk_weights(h):
                wqh = work.tile([P, KD, P], F8, name=f"wqh{h}", tag="wqh")
                nc.sync.dma_start(out=wqh.rearrange("p kd f -> p (kd f)"),
                                  in_=wqT_d[h])
                wkh = work.tile([P, KD, P], F8, name=f"wkh{h}", tag="wkh")
                nc.sync.dma_start(out=wkh.rearrange("p kd f -> p (kd f)"),
                                  in_=wkT_d[h])
                return wqh, wkh

            def alloc_head_tiles(h):
                vT = work.tile([P, T], BF16, name=f"vT{h}", tag="vT")
                vh = work.tile([P, TT, P], F8, name=f"vh{h}", tag="vh")
                vb0 = work.tile([P, P], BF16, name=f"vb0_{h}", tag="vb0")
                qt_ = work.tile([P, T], BF16, name=f"qt{h}", tag="qt")
                kt_ = work.tile([P, T], BF16, name=f"kt{h}", tag="kt")
                return vT, vh, vb0, qt_, kt_

            def proj_fill_gen(ws, tiles):
                """V^T then Q^T then K^T projection chains (fp8 DoubleRow),
                then the bf16 early-token patch chains, yielding after every
                matmul so the caller can interleave them."""
                wqh, wkh, wvh = ws
                vT, vh, vb0, qt_, kt_ = tiles
                for wh, dst in ((wvh, vT), (wqh, qt_), (wkh, kt_)):
                    for tb in range(TB):
                        ts_ = slice(tb * 512, (tb + 1) * 512)
                        ps = psP.tile([P, 512], F32, name="psfill", tag="psp")
                        for k2 in range(KD // 2):
                            nc.tensor.matmul(ps,
                                             lhsT=wh[:, 2 * k2:2 * k2 + 2, :],
                                             rhs=xt8[k2][:, :, ts_],
                                             start=(k2 == 0),
                                             stop=(k2 == KD // 2 - 1),
                                             perf_mode=DR)
                            yield
                        nc.vector.tensor_copy(out=dst[:, ts_], in_=ps)
                        if tb == 0:
                            # early-token patch: recompute tokens 0..127 from
                            # bf16 x (mixed-dtype: fp8 sign weights x bf16
                            # rhs), overwriting the fp8-derived values right
                            # after the tb0 eviction so the patch lands well
                            # before the head barrier (its eviction otherwise
                            # stalls the first S matmul of the next head).
                            ps2 = psP.tile([P, 512], F32, name="psfill",
                                           tag="psp")
                            for kd in range(KD):
                                nc.tensor.matmul(ps2[:, 0:P],
                                                 lhsT=wh[:, kd, :],
                                                 rhs=xtb[:, kd, :],
                                                 start=(kd == 0),
                                                 stop=(kd == KD - 1))
                                yield
                            nc.vector.tensor_copy(out=dst[:, 0:P],
                                                  in_=ps2[:, 0:P])

            def pump(gen, n):
                for _ in range(n):
                    try:
                        next(gen)
                    except StopIteration:
                        return False
                return True

            def pump_n(gen, n):
                c = 0
                for _ in range(n):
                    try:
                        next(gen)
                        c += 1
                    except StopIteration:
                        break
                return c

            def oproj_nb_gen(nb):
                """o_proj chains for one token block (needs all heads' ot
                columns of that block only), yielding per matmul."""
                ns = slice(nb * 512, (nb + 1) * 512)
                for m in range(MT):
                    ps = psP.tile([P, 512], F32, name="psout", tag="psp")
                    # heads 0,1: one fp8 DoubleRow matmul replaces two bf16
                    nc.tensor.matmul(ps, lhsT=wo8all[:, m, :, :],
                                     rhs=ot8[:, :, ns],
                                     start=True, stop=False, perf_mode=DR,
                                     skip_group_check=True)
                    yield
                    for hh in range(2, H):
                        nc.tensor.matmul(ps, lhsT=woall[:, m, hh, :],
                                         rhs=ot[hh][:, ns],
                                         start=False, stop=(hh == H - 1),
                                         skip_group_check=True)
                        yield
                    stg = work.tile([P, 512], F32, name="ostage", tag="ostage",
                                    bufs=4)
                    nc.vector.tensor_copy(out=stg, in_=ps)
                    nc.sync.dma_start(out=out_d[m * P:(m + 1) * P, ns],
                                      in_=stg)

            # head-0 Q/K weights and head-1 weights load after xt (the V^T
            # chains consume xt first; the Q chains run ~4 chain-times later)
            ws_list = [None] * (H + 2)
            wqh0, wkh0 = load_qk_weights(0)
            ws_list[0] = (wqh0, wkh0, wvh0)
            if H > 1:
                ws_list[1] = load_head_weights(1)
            cur_tiles = alloc_head_tiles(0)
            g0 = proj_fill_gen(ws_list[0], cur_tiles)
            while pump(g0, 1):
                pass

            fills = []

            def pump_fills(n):
                while n > 0 and fills:
                    n -= pump_n(fills[0], n)
                    if n > 0:
                        fills.pop(0)

            # o_proj weights are SBUF-resident: 64 per-(block,m-tile) weight
            # DMAs during the last head's attention saturate the Sync queue
            # (which also carries the output writes). Loaded around head 2,
            # when the DMA queue is otherwise quiet.
            woall = big.tile([P, MT, H, P], BF16, name="woall", tag="woall",
                             bufs=1)
            wo8all = big.tile([P, MT, 2, P], F8, name="wo8all", tag="wo8all",
                              bufs=1)

            for h in range(H):
                if h == 2:
                    for m in range(MT):
                        nc.sync.dma_start(
                            out=woall[:, m, :, :].rearrange("p h f -> p (h f)"),
                            in_=woT_d[m])
                        nc.sync.dma_start(
                            out=wo8all[:, m, :, :].rearrange("p h f -> p (h f)"),
                            in_=woT8_d[m])
                vT, vh, vb0, qt_, kt_ = cur_tiles
                # prefetch weights two heads ahead so fill matmuls never
                # wait on their DMA (a blocked fill stalls the in-order PE)
                if h + 2 < H:
                    ws_list[h + 2] = load_head_weights(h + 2)
                if h + 1 < H:
                    next_tiles = alloc_head_tiles(h + 1)
                    fills.append(proj_fill_gen(ws_list[h + 1], next_tiles))
                else:
                    next_tiles = None

                def emit_transpose(kt):
                    # lives in the psS pool: psP slots are held long by
                    # in-flight interleaved fill chains
                    pst = psS.tile([P, 512], BF16, name="pst", tag="pss")
                    nc.tensor.transpose(pst[:, 0:P],
                                        vT[:, kt * P:(kt + 1) * P], ident)
                    nc.vector.tensor_copy(out=vh[:, kt, :], in_=pst[:, 0:P])
                    if kt == 0:
                        # bf16 copy of key-tile 0 for the qb0 precision patch
                        nc.vector.tensor_copy(out=vb0, in_=pst[:, 0:P])

                # causal attention, S^T layout (keys on partitions).
                # Off-diagonal key tiles are paired into fp8 DoubleRow PV
                # matmuls; diagonal tiles (kt = 4*qb+di) only contribute to
                # query columns >= 128*di of the block and stay narrowed fp8
                # singles. The qb0/kt0 tile runs fully in bf16 (early-token
                # precision). Only the first 128 columns of a (narrowed)
                # diagonal tile are triangular; the rest are fully allowed.
                for qb in range(TB):
                    nkt = KT_PER_B * (qb + 1)
                    for kt in range(KT_PER_B * qb, nkt):
                        emit_transpose(kt)
                    psO_t = psO.tile([P, 512], F32, name="psodt", tag="pso")
                    # fp16 row-sum accumulator: 16-bit DVE ops run 2x, and
                    # fp16 is a valid matmul dtype so the ones-reduction can
                    # consume it directly (no bf16 staging copy). ~0.05% per
                    # add, ~0.2% on the normalizer over 16 adds: negligible.
                    racc = work.tile([P, 512], FP16, name="racc", tag="racc")
                    # off-diagonal pairs (kt in [0, 4qb), even count)
                    for pr in range(KT_PER_B * qb // 2):
                        ptp = work.tile([P, 2, 512], F8, name="ptp",
                                        tag="ptp", bufs=6)
                        for i in (0, 1):
                            kt = 2 * pr + i
                            psS_t = psS.tile([P, 512], F32, name="pssc",
                                             tag="pss")
                            nc.tensor.matmul(psS_t,
                                             lhsT=kt_[:, kt * P:(kt + 1) * P],
                                             rhs=qt_[:, qb * 512:(qb + 1) * 512],
                                             start=True, stop=True)
                            nc.scalar.activation(
                                out=ptp[:, i, :], in_=psS_t,
                                func=mybir.ActivationFunctionType.Exp,
                                scale=qks)
                            if kt == 0:
                                nc.vector.tensor_copy(out=racc,
                                                      in_=ptp[:, 0, :])
                            else:
                                nc.vector.tensor_add(racc, racc,
                                                     ptp[:, i, :])
                            pump_fills(2)
                        nc.tensor.matmul(psO_t, lhsT=vh[:, 2 * pr:2 * pr + 2, :],
                                         rhs=ptp,
                                         start=(pr == 0), stop=False,
                                         perf_mode=DR, skip_group_check=True)
                        pump_fills(1)
                    # diagonal singles
                    for di in range(KT_PER_B):
                        kt = KT_PER_B * qb + di
                        c0 = di * P  # first live query column
                        w = 512 - c0
                        qs = slice(qb * 512 + c0, (qb + 1) * 512)
                        psS_t = psS.tile([P, 512], F32, name="pssc", tag="pss")
                        nc.tensor.matmul(psS_t[:, :w],
                                         lhsT=kt_[:, kt * P:(kt + 1) * P],
                                         rhs=qt_[:, qs],
                                         start=True, stop=True)
                        if qb == 0 and kt == 0:
                            pt0 = work.tile([P, 512], BF16, name="pt0",
                                            tag="pt0", bufs=1)
                            nc.scalar.activation(
                                out=pt0, in_=psS_t,
                                func=mybir.ActivationFunctionType.Exp,
                                scale=qks)
                            nc.vector.tensor_mul(pt0[:, :P], pt0[:, :P], cmb)
                            pump_fills(2)  # PE fill work while exp runs
                            nc.tensor.matmul(psO_t, lhsT=vb0, rhs=pt0,
                                             start=True, stop=(di == KT_PER_B - 1),
                                             skip_group_check=True)
                            nc.vector.tensor_copy(out=racc, in_=pt0)
                        else:
                            pt = work.tile([P, 512], F8, name="pexp", tag="pt",
                                           bufs=8)
                            nc.scalar.activation(
                                out=pt[:, :w], in_=psS_t[:, :w],
                                func=mybir.ActivationFunctionType.Exp,
                                scale=qks)
                            # first 128 cols of every narrowed diagonal
                            # window are triangular
                            nc.vector.tensor_mul(pt[:, :P], pt[:, :P],
                                                 cmask[:, 0, :P])
                            pump_fills(2)  # PE fill work while exp runs
                            nc.tensor.matmul(psO_t[:, c0:], lhsT=vh[:, kt, :],
                                             rhs=pt[:, :w],
                                             start=(qb == 0 and di == 0),
                                             stop=(di == KT_PER_B - 1),
                                             skip_group_check=True)
                            if kt == 0:
                                nc.vector.tensor_copy(out=racc, in_=pt)
                            else:
                                nc.vector.tensor_add(racc[:, c0:], racc[:, c0:],
                                                     pt[:, :w])
                        pump_fills(di & 1)
                    psR_t = psR.tile([P, 512], F32, name="psrow", tag="psr")
                    nc.tensor.matmul(psR_t, lhsT=(onesgv if h < 2 else ones),
                                     rhs=racc, start=True, stop=True)
                    rec = work.tile([P, 512], F32, name="rec", tag="rec")
                    nc.vector.reciprocal_approx_fast(out=rec, in_=psR_t)
                    if h < 2:
                        nc.vector.tensor_mul(
                            ot8[:, h, qb * 512:(qb + 1) * 512], psO_t, rec)
                    else:
                        nc.vector.tensor_mul(
                            ot[h][:, qb * 512:(qb + 1) * 512], psO_t, rec)
                    if h == H - 1:
                        # this token block's ot columns are now complete for
                        # every head: its o_proj chains become fill work
                        fills.append(oproj_nb_gen(qb))
                    pump_fills(4)
                if h < H - 1:
                    # finish next head's projections before its attention
                    while fills:
                        pump_fills(64)
                cur_tiles = next_tiles
            # drain remaining o_proj work
            while fills:
                pump_fills(64)

    nc.compile()
    return nc


def _bitlinear_sign_gamma(w):
    """BitLinear split: exact ternary sign pattern and the gamma scale."""
    w = np.asarray(w, dtype=np.float32)
    gamma = max(np.float32(np.abs(w).mean(dtype=np.float32)),
                np.float32(1e-5))
    sign = np.clip(np.round(w / gamma), -1.0, 1.0).astype(np.float32)
    return sign, gamma


def _causal_masks():
    k = np.arange(128)[:, None]
    q = np.arange(512)[None, :]
    m = np.stack([(k <= q - 128 * i) for i in range(4)]).astype(np.float32)
    return m


def _tile_qkv(w_shard):
    """[F, D] -> [H, 128, KD*128]: [h, p, kd*128+f] = w_shard[h*128+f, kd*128+p]."""
    Fs, Ds = w_shard.shape
    a = w_shard.reshape(Fs // 128, 128, Ds // 128, 128)  # [h, f, kd, p]
    a = a.transpose(0, 3, 2, 1).reshape(Fs // 128, 128, Ds)
    return np.ascontiguousarray(a)


def _tile_wo(wo_shard):
    """[D, F] -> [MT, 128, H*128]: [m, p, h*128+j] = wo_shard[m*128+j, h*128+p]."""
    Ds, Fs = wo_shard.shape
    a = wo_shard.reshape(Ds // 128, 128, Fs // 128, 128)  # [m, j, h, p]
    a = a.transpose(0, 3, 2, 1).reshape(Ds // 128, 128, Fs)
    return np.ascontiguousarray(a)


def _prep_inputs(x, wq, wk, wv, wo):
    bf = ml_dtypes.bfloat16
    f8 = ml_dtypes.float8_e4m3fn
    x = np.asarray(x, dtype=np.float32)
    sq, gq = _bitlinear_sign_gamma(wq)
    sk, gk = _bitlinear_sign_gamma(wk)
    sv, gv = _bitlinear_sign_gamma(wv)
    so, go = _bitlinear_sign_gamma(wo)
    cm = _causal_masks()
    cmask = cm.astype(f8)
    cmb = np.ascontiguousarray(cm[0][:, :128]).astype(bf)
    qks = np.full((128, 1), gq * gk / math.sqrt(D_HEAD), dtype=np.float32)
    onesgv = np.full((128, 128), 1.0 / (go * gv * 64.0),
                     dtype=np.float32).astype(bf)
    so_8 = so * np.float32(2.0 ** -6)
    xT8s = [np.ascontiguousarray(x[b].T).astype(f8) for b in range(B)]
    xTbs = [np.ascontiguousarray(x[b].T[:, :128]).astype(bf) for b in range(B)]
    wo_scaled = so * (go * gv)
    shards = []
    for hg in range(2):
        rows = slice(hg * F_LOC, (hg + 1) * F_LOC)
        shards.append({
            "wqT": _tile_qkv(sq[rows, :]).astype(f8),
            "wkT": _tile_qkv(sk[rows, :]).astype(f8),
            "wvT": _tile_qkv(sv[rows, :]).astype(f8),
            "woT": _tile_wo(wo_scaled[:, rows]).astype(bf),
            "woT8": _tile_wo(
                so_8[:, hg * F_LOC:hg * F_LOC + 256]).astype(f8),
        })
    in_maps = []
    for c in range(N_CORES):
        b, hg = c // 2, c % 2
        m = {"xT8": xT8s[b], "xTb": xTbs[b], "cmask": cmask, "cmb": cmb,
             "qks": qks, "onesgv": onesgv}
        m.update(shards[hg])
        in_maps.append(m)
    return in_maps


_NC_CACHE = {}


def _get_nc():
    if "nc" not in _NC_CACHE:
        _NC_CACHE["nc"] = build_bass()
    return _NC_CACHE["nc"]


def run(x, wq, wk, wv, wo, trace=False):
    nc = _get_nc()
    in_maps = _prep_inputs(x, wq, wk, wv, wo)
    res = bass_utils.run_bass_kernel_spmd(
        nc, in_maps, core_ids=list(range(N_CORES)), trace=trace)
    out = np.empty((B, T_FULL, D_MODEL), dtype=np.float32)
    for b in range(B):
        out[b] = (res.results[2 * b]["outT"]
                  + res.results[2 * b + 1]["outT"]).T
    return out, res


def kernel(x, wq, wk, wv, wo):
    out, _ = run(x, wq, wk, wv, wo)
    return out
